# revision 3
# baseline (speedup 1.0000x reference)
"""Trainium2 Bass kernel for nn_BPDecoder: logits = 1 - exp(-exp(sum_i R_i*||Z_i||^2)).

v3 strategy (8-core SPMD, row-sharded):
  - Error budget: logits tol 2e-2 allows ~30% rel error on the scalar s, so
    Z rides the wire in fp8 e4m3 (host scales by 512; /512^2 on host at the
    end).  Measured end-to-end err ~2e-3.
  - Row r of a core maps to (partition, tile, q) = (r//496, (r%496)//16,
    r%16): the wire buffer is literally Zfp8.reshape(128, 63488) and every
    DMA slab z[:, c0:c1] is contiguous per partition.
  - Two square pipelines:
      * ACT tiles (12, grouped in 6 adjacent pairs): plain fp8 DMA on the
        SP HWDGE ring; ACT squares fp8->fp8 (~2.0us/tile); PE contracts
        each pair with fp8 DoubleRow matmuls (256 rows, 2 cols/cycle).
      * DVE tiles (19): gpsimd SWDGE *casting* DMA lands them as bf16 in
        SBUF (same fp8 wire bytes); DVE squares bf16 in-place in 2x_1p
        mode (~1.15us/tile); PE runs plain bf16 matmuls.
    gpsimd never computes (Q7 tensor ops trigger the power throttle that
    halves DVE throughput -- measured); it only generates SWDGE descriptors.
  - PSUM: 4 banks accumulate the DoubleRow group, 4 banks the plain group.
    The DR group closes early and its banks are copied out by ACT while DVE
    matmuls still run; host sums the q'==q diagonal blocks of both groups
    and applies 1 - exp(-exp(s)) in f64.
"""

import sys

sys.path.insert(0, "/opt/trn_rl_repo")


# The agent image lacks antenv.axon_hooks; recreate it so trace=True works
# (bass_utils imports it lazily for NTFF profiling under axon).
def _install_ntff_hook_shim():
    import types
    if "antenv.axon_hooks" in sys.modules:
        return
    mod = types.ModuleType("antenv.axon_hooks")
    state = {"hook": None}
    mod.set_axon_ntff_profile_hook = lambda h: state.__setitem__("hook", h)
    mod.get_axon_ntff_profile_hook = lambda: state["hook"]
    sys.modules["antenv.axon_hooks"] = mod
    try:
        sys.path.insert(0, "/root/.axon_site")
        from trn_agent_boot.trn_boot import _ntff_profile_via_ctypes
        state["hook"] = _ntff_profile_via_ctypes("/opt/axon/libaxon_pjrt.so")
    except Exception:
        pass


_install_ntff_hook_shim()

import numpy as np

import concourse.bass as bass
import concourse.bacc as bacc
import concourse.mybir as mybir
from concourse.tile import TileContext
from concourse.bass_utils import run_bass_kernel_spmd

P = 128          # SBUF partitions
D = 128          # row length (feature dim)
Q = 16           # rows per partition per tile
T = 31           # tiles per core
FREE = Q * D     # 2048 free elems per partition per tile
ROWS_PER_PART = T * Q         # 496
NC_ROWS = P * ROWS_PER_PART   # 63488 rows per core
N_CORES = 8
N_FULL = 500000
MM_N = 512       # matmul moving slice (PSUM bank: 512 f32 per partition)
NSLICES = FREE // MM_N        # 4

Z_DT = mybir.dt.float8e4
Z_SCALE_IN = 512.0            # host multiplies Z by this before the fp8 cast

# ACT handles these tiles as DoubleRow pairs (adjacent tiles); DVE gets the rest
ACT_PAIRS = [(2, 3), (7, 8), (12, 13), (16, 17), (20, 21), (24, 25)]
ACT_TILES = sorted(t for p in ACT_PAIRS for t in p)
DVE_TILES = [t for t in range(T) if t not in ACT_TILES]
NPAIRS = len(ACT_PAIRS)       # 6
NPLAIN = len(DVE_TILES)       # 19

# DMA slabs: (c0, c1, kind) covering tile ranges homogeneously
_slabs = []
_t = 0
while _t < T:
    kind = "act" if _t in ACT_TILES else "dve"
    t1 = _t
    while t1 < T and (("act" if t1 in ACT_TILES else "dve") == kind):
        t1 += 1
    # split very long runs so arrival granularity stays fine
    while t1 - _t > 3:
        _slabs.append((_t, _t + 3, kind))
        _t += 3
    _slabs.append((_t, t1, kind))
    _t = t1
SLABS = _slabs

_cache = {}


def _np_dt(dt):
    return mybir.dt.np(dt)


def _build():
    nc = bacc.Bacc(trn_type="TRN2")
    z = nc.declare_dram_parameter("z", [P, T * FREE], Z_DT, isOutput=False)
    r8 = nc.declare_dram_parameter("r8", [P, NPAIRS * 2 * Q], Z_DT, isOutput=False)
    rb = nc.declare_dram_parameter("rb", [P, NPLAIN * Q], mybir.dt.bfloat16,
                                   isOutput=False)
    out = nc.declare_dram_parameter("out", [Q, 2 * FREE], mybir.dt.float32,
                                    isOutput=True)

    act_idx = {t: i for i, t in enumerate(ACT_TILES)}
    dve_idx = {t: i for i, t in enumerate(DVE_TILES)}

    with TileContext(nc) as tc:
        with (
            tc.tile_pool(name="singles", bufs=1) as singles,
            tc.tile_pool(name="ppool", bufs=1, space="PSUM") as ppool,
        ):
            r8_sb = singles.tile([P, NPAIRS, 2, Q], Z_DT)
            rb_sb = singles.tile([P, NPLAIN, Q], mybir.dt.bfloat16)
            nc.sync.dma_start(out=r8_sb[:], in_=r8[:])
            nc.sync.dma_start(out=rb_sb[:], in_=rb[:])

            z8_sb = singles.tile([P, len(ACT_TILES), FREE], Z_DT)
            zb_sb = singles.tile([P, NPLAIN, FREE], mybir.dt.bfloat16)

            for (c0, c1, kind) in SLABS:
                src = z[:, c0 * FREE:c1 * FREE]
                if kind == "act":
                    j0, j1 = act_idx[c0], act_idx[c1 - 1] + 1
                    nc.sync.dma_start(out=z8_sb[:, j0:j1, :], in_=src)
                else:
                    j0, j1 = dve_idx[c0], dve_idx[c1 - 1] + 1
                    nc.gpsimd.dma_start(out=zb_sb[:, j0:j1, :], in_=src)

            sq8_sb = singles.tile([P, len(ACT_TILES), FREE], Z_DT)

            # squares: ACT fp8->fp8 into sq8_sb; DVE bf16 in-place (2x_1p)
            for t in range(T):
                if t in act_idx:
                    j = act_idx[t]
                    nc.scalar.square(sq8_sb[:, j, :], z8_sb[:, j, :])
                else:
                    j = dve_idx[t]
                    nc.vector.tensor_mul(zb_sb[:, j, :], zb_sb[:, j, :],
                                         zb_sb[:, j, :])

            acc_dr = [ppool.tile([Q, MM_N], mybir.dt.float32, name=f"adr{i}")
                      for i in range(NSLICES)]
            acc_pl = [ppool.tile([Q, MM_N], mybir.dt.float32, name=f"apl{i}")
                      for i in range(NSLICES)]

            # matmul units in tile-readiness order
            units = []   # (sortkey, kind, payload)
            for k, (ta, tb) in enumerate(ACT_PAIRS):
                units.append((tb, "dr", k))
            for t in DVE_TILES:
                units.append((t, "pl", dve_idx[t]))
            units.sort()

            ndr = npl = 0
            for _, kind, j in units:
                if kind == "dr":
                    for sl in range(NSLICES):
                        nc.tensor.matmul(
                            acc_dr[sl][:],
                            r8_sb[:, j, :, :],
                            sq8_sb[:, 2 * j:2 * j + 2, sl * MM_N:(sl + 1) * MM_N],
                            start=(ndr == 0),
                            stop=(ndr == NPAIRS - 1),
                            perf_mode=mybir.MatmulPerfMode.DoubleRow,
                        )
                    ndr += 1
                else:
                    for sl in range(NSLICES):
                        nc.tensor.matmul(
                            acc_pl[sl][:],
                            rb_sb[:, j, :],
                            zb_sb[:, j, sl * MM_N:(sl + 1) * MM_N],
                            start=(npl == 0),
                            stop=(npl == NPLAIN - 1),
                        )
                    npl += 1

            out_sb = singles.tile([Q, 2 * FREE], mybir.dt.float32)
            # DR banks close early -> ACT copies them while DVE matmuls run
            for sl in range(NSLICES):
                nc.scalar.copy(out_sb[:, sl * MM_N:(sl + 1) * MM_N], acc_dr[sl][:])
            for sl in range(NSLICES):
                copy_eng = nc.vector.tensor_copy if sl % 2 == 0 else nc.scalar.copy
                copy_eng(out_sb[:, FREE + sl * MM_N:FREE + (sl + 1) * MM_N],
                         acc_pl[sl][:])
            nc.sync.dma_start(out=out[:], in_=out_sb[:])
    nc.compile()
    return nc


def _get_nc():
    if "nc" not in _cache:
        _cache["nc"] = _build()
    return _cache["nc"]


def _shard(Z, R):
    np_z = _np_dt(Z_DT)
    np_bf = _np_dt(mybir.dt.bfloat16)
    ZP = np.zeros((N_CORES * NC_ROWS, D), dtype=np_z)
    ZP[:N_FULL] = (Z * np.float32(Z_SCALE_IN)).astype(np_z)
    ZW = ZP.reshape(N_CORES, P, T * FREE)

    RP = np.zeros((N_CORES * NC_ROWS,), dtype=np.float32)
    RP[:N_FULL] = R
    RV = RP.reshape(N_CORES, P, T, Q)

    R8 = np.zeros((N_CORES, P, NPAIRS, 2, Q), dtype=np.float32)
    for k, (ta, tb) in enumerate(ACT_PAIRS):
        R8[:, :, k, 0] = RV[:, :, ta]
        R8[:, :, k, 1] = RV[:, :, tb]
    R8 = np.ascontiguousarray(R8.astype(np_z)).reshape(N_CORES, P, NPAIRS * 2 * Q)

    RB = np.zeros((N_CORES, P, NPLAIN, Q), dtype=np.float32)
    for j, t in enumerate(DVE_TILES):
        RB[:, :, j] = RV[:, :, t]
    RB = np.ascontiguousarray(RB.astype(np_bf)).reshape(N_CORES, P, NPLAIN * Q)

    return [{"z": ZW[kk], "r8": R8[kk], "rb": RB[kk]} for kk in range(N_CORES)]


def _combine(results):
    idx = np.arange(Q)
    s = 0.0
    for res in results:
        C = np.asarray(res["out"], dtype=np.float64).reshape(Q, 2, Q, D)
        s += C[idx, :, idx, :].sum()
    s /= float(Z_SCALE_IN) ** 2
    lam = np.exp(s)
    logits = 1.0 - np.exp(-lam)
    return np.float32(logits)


def _run(Z, R, trace=False, tmpdir=None):
    nc = _get_nc()
    in_maps = _shard(Z, R)
    return run_bass_kernel_spmd(nc, in_maps, core_ids=list(range(N_CORES)),
                                trace=trace, tmpdir=tmpdir)


def kernel(Z, R):
    assert Z.shape == (N_FULL, D) and R.shape == (N_FULL,)
    out = _run(np.asarray(Z), np.asarray(R), trace=False)
    return _combine(out.results)


# revision 4
# speedup vs baseline: 1.0309x; 1.0309x over previous
"""Trainium2 Bass kernel for nn_BPDecoder: logits = 1 - exp(-exp(sum_i R_i*||Z_i||^2)).

v4 (8-core SPMD, row-sharded, LP-balanced engine mix, big-descriptor DMA):
  - fp8 e4m3 wire for Z (host scales by 512; /512^2 at the end).  Row r of a
    core maps to (partition, tile, q) = (r//496, (r%496)//16, r%16): the wire
    buffer is Zfp8.reshape(128, 63488); slabs are per-partition contiguous
    runs of 2-10KB, which the DMA engines move at ~385GB/s (measured; 2KB
    runs only reach ~295GB/s).
  - 31 tiles split:
      * BF16_TILES (9, three batches of 3): gpsimd SWDGE *casting* DMAs land
        them as bf16; DVE squares each batch in-place with one 2x_1p
        tensor_mul (~1.15us/tile); PE runs plain bf16 matmuls into their own
        PSUM group (closes before the fp8 group).
      * 22 fp8 tiles -> 11 DoubleRow pairs (adjacent slots in arrival
        order).  Each pair is squared by ONE 2-tile instruction on ACT
        (square, ~1.85us/tile) or DVE (tensor_mul, ~2.21us/tile), assigned
        per-pair by a greedy schedule against DMA arrival estimates; the PE
        contracts each pair with fp8 DoubleRow matmuls (256 rows,
        2 cols/cycle).
  - gpsimd never computes (Q7 tensor ops trip a power throttle that halves
    DVE throughput -- measured); it only generates SWDGE descriptors.
  - Host extracts the q'==q diagonal blocks of both groups and applies
    1 - exp(-exp(s)) in f64.
"""

import sys

sys.path.insert(0, "/opt/trn_rl_repo")


def _install_ntff_hook_shim():
    import types
    if "antenv.axon_hooks" in sys.modules:
        return
    mod = types.ModuleType("antenv.axon_hooks")
    state = {"hook": None}
    mod.set_axon_ntff_profile_hook = lambda h: state.__setitem__("hook", h)
    mod.get_axon_ntff_profile_hook = lambda: state["hook"]
    sys.modules["antenv.axon_hooks"] = mod
    try:
        sys.path.insert(0, "/root/.axon_site")
        from trn_agent_boot.trn_boot import _ntff_profile_via_ctypes
        state["hook"] = _ntff_profile_via_ctypes("/opt/axon/libaxon_pjrt.so")
    except Exception:
        pass


_install_ntff_hook_shim()

import numpy as np

import concourse.bass as bass
import concourse.bacc as bacc
import concourse.mybir as mybir
from concourse.tile import TileContext
from concourse.bass_utils import run_bass_kernel_spmd

P = 128
D = 128
Q = 16
T = 31
FREE = Q * D                  # 2048
ROWS_PER_PART = T * Q         # 496
NC_ROWS = P * ROWS_PER_PART   # 63488
N_CORES = 8
N_FULL = 500000
MM_N = 512
NSLICES = FREE // MM_N        # 4

Z_DT = mybir.dt.float8e4
Z_SCALE_IN = 512.0

# three cast batches of 3 tiles each
CAST_SLABS = [(1, 4), (11, 14), (19, 22)]
BF16_TILES = [t for (c0, c1) in CAST_SLABS for t in range(c0, c1)]
FP8_TILES = [t for t in range(T) if t not in BF16_TILES]   # 22 tiles
NPAIRS = len(FP8_TILES) // 2                               # 11
NPLAIN = len(BF16_TILES)                                   # 9
assert len(FP8_TILES) == 2 * NPAIRS

# SP slabs covering the fp8 runs {0}, {4..10}, {14..18}, {22..30}
SP_SLABS = [(0, 1), (4, 8), (8, 11), (14, 19), (22, 26), (26, 29), (29, 31)]
assert sorted(t for c0, c1 in SP_SLABS for t in range(c0, c1)) == FP8_TILES


def _schedule():
    """Arrival estimates (us from profile t0) and per-pair engine greedy."""
    rate = {"sp": 0.70, "cast": 1.40}     # us per tile of DMA-engine time
    events = [("sp", SP_SLABS[0]), ("cast", CAST_SLABS[0]),
              ("sp", SP_SLABS[1]), ("sp", SP_SLABS[2]),
              ("cast", CAST_SLABS[1]), ("sp", SP_SLABS[3]),
              ("cast", CAST_SLABS[2]), ("sp", SP_SLABS[4]),
              ("sp", SP_SLABS[5]), ("sp", SP_SLABS[6])]
    arr = {}
    tdone = 10.5
    for kind, (c0, c1) in events:
        tdone += rate[kind] * (c1 - c0)
        for t in range(c0, c1):
            arr[t] = tdone
    # DVE pre-load: bf16 batches (one 3-tile instr each, ~3.4us)
    dve_busy = []
    for (c0, c1) in CAST_SLABS:
        dve_busy.append((arr[c1 - 1], 3.4))
    pair_engine = []
    free = {"act": 0.0, "dve": 0.0}
    cost = {"act": 3.71, "dve": 4.42}

    def dve_adjusted(t0):
        # push DVE start past any bf16 batch that lands before t0
        s = t0
        for (a, dur) in dve_busy:
            if a <= s:
                s += dur
        return s

    for k in range(NPAIRS):
        ready = arr[FP8_TILES[2 * k + 1]]
        fin = {}
        fin["act"] = max(ready, free["act"]) + cost["act"]
        fin["dve"] = max(ready, dve_adjusted(free["dve"])) + cost["dve"]
        e = min(("act", "dve"), key=lambda e: (fin[e], cost[e]))
        pair_engine.append(e)
        free[e] = fin[e]
    return pair_engine


PAIR_ENGINE = _schedule()

_cache = {}


def _np_dt(dt):
    return mybir.dt.np(dt)


def _build():
    nc = bacc.Bacc(trn_type="TRN2")
    z = nc.declare_dram_parameter("z", [P, T * FREE], Z_DT, isOutput=False)
    r8 = nc.declare_dram_parameter("r8", [P, NPAIRS * 2 * Q], Z_DT, isOutput=False)
    rb = nc.declare_dram_parameter("rb", [P, NPLAIN * Q], mybir.dt.bfloat16,
                                   isOutput=False)
    out = nc.declare_dram_parameter("out", [Q, 2 * FREE], mybir.dt.float32,
                                    isOutput=True)

    slot = {t: j for j, t in enumerate(FP8_TILES)}
    bslot = {t: j for j, t in enumerate(BF16_TILES)}

    with TileContext(nc) as tc:
        with (
            tc.tile_pool(name="singles", bufs=1) as singles,
            tc.tile_pool(name="ppool", bufs=1, space="PSUM") as ppool,
        ):
            r8_sb = singles.tile([P, NPAIRS, 2, Q], Z_DT)
            rb_sb = singles.tile([P, NPLAIN, Q], mybir.dt.bfloat16)
            nc.sync.dma_start(out=r8_sb[:], in_=r8[:])
            nc.sync.dma_start(out=rb_sb[:], in_=rb[:])

            z8_sb = singles.tile([P, len(FP8_TILES), FREE], Z_DT)
            zb_sb = singles.tile([P, NPLAIN, FREE], mybir.dt.bfloat16)
            sq8_sb = singles.tile([P, len(FP8_TILES), FREE], Z_DT)

            for (c0, c1) in CAST_SLABS:
                j0, j1 = bslot[c0], bslot[c1 - 1] + 1
                nc.gpsimd.dma_start(out=zb_sb[:, j0:j1, :],
                                    in_=z[:, c0 * FREE:c1 * FREE])
            for (c0, c1) in SP_SLABS:
                j0, j1 = slot[c0], slot[c1 - 1] + 1
                nc.sync.dma_start(out=z8_sb[:, j0:j1, :],
                                  in_=z[:, c0 * FREE:c1 * FREE])

            # squares: one instruction per DR pair / per bf16 batch, emitted
            # in arrival order
            sq_units = []
            for bi, (c0, c1) in enumerate(CAST_SLABS):
                sq_units.append((c1 - 1, "bf", bi))
            for k in range(NPAIRS):
                sq_units.append((FP8_TILES[2 * k + 1], "p8", k))
            sq_units.sort()
            for _, kind, j in sq_units:
                if kind == "bf":
                    c0, c1 = CAST_SLABS[j]
                    j0, j1 = bslot[c0], bslot[c1 - 1] + 1
                    nc.vector.tensor_mul(zb_sb[:, j0:j1, :], zb_sb[:, j0:j1, :],
                                         zb_sb[:, j0:j1, :])
                else:
                    sl2 = slice(2 * j, 2 * j + 2)
                    if PAIR_ENGINE[j] == "act":
                        nc.scalar.square(sq8_sb[:, sl2, :], z8_sb[:, sl2, :])
                    else:
                        nc.vector.tensor_mul(sq8_sb[:, sl2, :], z8_sb[:, sl2, :],
                                             z8_sb[:, sl2, :])

            acc_dr = [ppool.tile([Q, MM_N], mybir.dt.float32, name=f"adr{i}")
                      for i in range(NSLICES)]
            acc_pl = [ppool.tile([Q, MM_N], mybir.dt.float32, name=f"apl{i}")
                      for i in range(NSLICES)]

            units = []
            for j, t in enumerate(BF16_TILES):
                units.append((t, 1, "pl", j))
            for k in range(NPAIRS):
                units.append((FP8_TILES[2 * k + 1], 0, "dr", k))
            units.sort()

            ndr = npl = 0
            for _, _, kind, j in units:
                if kind == "dr":
                    for sl in range(NSLICES):
                        nc.tensor.matmul(
                            acc_dr[sl][:],
                            r8_sb[:, j, :, :],
                            sq8_sb[:, 2 * j:2 * j + 2, sl * MM_N:(sl + 1) * MM_N],
                            start=(ndr == 0),
                            stop=(ndr == NPAIRS - 1),
                            perf_mode=mybir.MatmulPerfMode.DoubleRow,
                        )
                    ndr += 1
                else:
                    for sl in range(NSLICES):
                        nc.tensor.matmul(
                            acc_pl[sl][:],
                            rb_sb[:, j, :],
                            zb_sb[:, j, sl * MM_N:(sl + 1) * MM_N],
                            start=(npl == 0),
                            stop=(npl == NPLAIN - 1),
                        )
                    npl += 1

            out_sb = singles.tile([Q, 2 * FREE], mybir.dt.float32)
            # plain group closes first: DVE copies it, SP ships it mid-run
            for sl in range(NSLICES):
                nc.vector.tensor_copy(out_sb[:, FREE + sl * MM_N:
                                             FREE + (sl + 1) * MM_N],
                                      acc_pl[sl][:])
            nc.sync.dma_start(out=out[:, FREE:], in_=out_sb[:, FREE:])
            for sl in range(NSLICES):
                copy_eng = nc.scalar.copy if sl % 2 == 0 else nc.vector.tensor_copy
                copy_eng(out_sb[:, sl * MM_N:(sl + 1) * MM_N], acc_dr[sl][:])
            nc.sync.dma_start(out=out[:, :FREE], in_=out_sb[:, :FREE])
    nc.compile()
    return nc


def _get_nc():
    if "nc" not in _cache:
        _cache["nc"] = _build()
    return _cache["nc"]


def _shard(Z, R):
    np_z = _np_dt(Z_DT)
    np_bf = _np_dt(mybir.dt.bfloat16)
    ZP = np.zeros((N_CORES * NC_ROWS, D), dtype=np_z)
    ZP[:N_FULL] = (Z * np.float32(Z_SCALE_IN)).astype(np_z)
    ZW = ZP.reshape(N_CORES, P, T * FREE)

    RP = np.zeros((N_CORES * NC_ROWS,), dtype=np.float32)
    RP[:N_FULL] = R
    RV = RP.reshape(N_CORES, P, T, Q)

    R8 = np.zeros((N_CORES, P, NPAIRS, 2, Q), dtype=np.float32)
    for k in range(NPAIRS):
        R8[:, :, k, 0] = RV[:, :, FP8_TILES[2 * k]]
        R8[:, :, k, 1] = RV[:, :, FP8_TILES[2 * k + 1]]
    R8 = np.ascontiguousarray(R8.astype(np_z)).reshape(N_CORES, P, NPAIRS * 2 * Q)

    RB = np.zeros((N_CORES, P, NPLAIN, Q), dtype=np.float32)
    for j, t in enumerate(BF16_TILES):
        RB[:, :, j] = RV[:, :, t]
    RB = np.ascontiguousarray(RB.astype(np_bf)).reshape(N_CORES, P, NPLAIN * Q)

    return [{"z": ZW[kk], "r8": R8[kk], "rb": RB[kk]} for kk in range(N_CORES)]


def _combine(results):
    idx = np.arange(Q)
    s = 0.0
    for res in results:
        C = np.asarray(res["out"], dtype=np.float64).reshape(Q, 2, Q, D)
        s += C[idx, :, idx, :].sum()
    s /= float(Z_SCALE_IN) ** 2
    lam = np.exp(s)
    logits = 1.0 - np.exp(-lam)
    return np.float32(logits)


def _run(Z, R, trace=False, tmpdir=None):
    nc = _get_nc()
    in_maps = _shard(Z, R)
    return run_bass_kernel_spmd(nc, in_maps, core_ids=list(range(N_CORES)),
                                trace=trace, tmpdir=tmpdir)


def kernel(Z, R):
    assert Z.shape == (N_FULL, D) and R.shape == (N_FULL,)
    out = _run(np.asarray(Z), np.asarray(R), trace=False)
    return _combine(out.results)


# revision 5
# speedup vs baseline: 1.3380x; 1.2979x over previous
"""Trainium2 Bass kernel for nn_BPDecoder: logits = 1 - exp(-exp(sum_i R_i*||Z_i||^2)).

v5 (8-core SPMD, row-sharded, DVE bit-arithmetic squares):
  - The logits tolerance (2e-2) allows ~30% relative error on the scalar s,
    which unlocks aggressive quantization.  Z rides the wire as |Z|*512 in
    fp8 e4m3 (sign dropped -- only squares are needed); the result is
    divided by 512^2 on host.
  - Row r of a core maps to (partition, tile, q) = (r//496, (r%496)//16,
    r%16): the wire buffer is exactly Zfp8.reshape(128, 63488), and DMA
    slabs are per-partition contiguous runs of 1.4-8KB (measured
    ~385GB/s with big descriptors on the SP HWDGE ring).
  - 31 tiles, one resident SBUF wire buffer, two square pipelines:
      * 6 ACT pairs: nc.scalar.square fp8->fp8 (one 2-tile instruction per
        pair, ~1.85us/tile); PE contracts each pair with fp8 DoubleRow
        matmuls (256 rows, 2 cols/cycle at full clock).
      * 19 DVE tiles: *bit-arithmetic square* -- for abs-fp8 bytes,
        bf16_bits(z^2) ~= 32*byte + 14470 (exponent doubles, mantissa
        affine fit; measured s-error ~2% of budget).  This is a uint8
        tensor_scalar (mult+add) which runs in the DVE's dtype-agnostic
        2x_2P mode: ~1.15us/tile, half the cost of tensor_mul.  The
        uint16 outputs ARE bf16 squares; the PE consumes them with plain
        bf16 matmuls into a second PSUM group.
  - gpsimd never computes and never issues DMAs (Q7 activity trips a power
    throttle that halves DVE throughput, and SWDGE cast-DMAs double the
    DMA-engine byte load -- both measured).
  - Host extracts the q'==q diagonal blocks of both output groups and
    applies 1 - exp(-exp(s)) in f64.
"""

import sys

sys.path.insert(0, "/opt/trn_rl_repo")


def _install_ntff_hook_shim():
    import types
    if "antenv.axon_hooks" in sys.modules:
        return
    mod = types.ModuleType("antenv.axon_hooks")
    state = {"hook": None}
    mod.set_axon_ntff_profile_hook = lambda h: state.__setitem__("hook", h)
    mod.get_axon_ntff_profile_hook = lambda: state["hook"]
    sys.modules["antenv.axon_hooks"] = mod
    try:
        sys.path.insert(0, "/root/.axon_site")
        from trn_agent_boot.trn_boot import _ntff_profile_via_ctypes
        state["hook"] = _ntff_profile_via_ctypes("/opt/axon/libaxon_pjrt.so")
    except Exception:
        pass


_install_ntff_hook_shim()

import numpy as np

import concourse.bass as bass
import concourse.bacc as bacc
import concourse.mybir as mybir
from concourse.tile import TileContext
from concourse.bass_utils import run_bass_kernel_spmd

P = 128
D = 128
Q = 16
T = 31
FREE = Q * D                  # 2048
ROWS_PER_PART = T * Q         # 496
NC_ROWS = P * ROWS_PER_PART   # 63488
N_CORES = 8
N_FULL = 500000
MM_N = 512
NSLICES = FREE // MM_N        # 4

Z_DT = mybir.dt.float8e4
Z_SCALE_IN = 512.0
SQ_K = 14470.0                # bf16_bits(z^2) ~= 32*abs_fp8_byte + SQ_K

# ACT squares these adjacent tile pairs (fp8 -> DoubleRow); DVE bit-squares
# the rest (bf16 -> plain matmuls).  Pairs are placed so each arrives just
# in time for ACT's ~3.7us/pair cadence.
ACT_PAIRS = [(0, 1), (4, 5), (9, 10), (14, 15), (19, 20), (24, 25)]
ACT_TILES = [t for p in ACT_PAIRS for t in p]
DVE_TILES = [t for t in range(T) if t not in ACT_TILES]
NPAIRS = len(ACT_PAIRS)       # 6
NPLAIN = len(DVE_TILES)       # 19

SLAB_SIZES = [2, 3, 4, 4, 4, 4, 4, 3, 2, 1]
assert sum(SLAB_SIZES) == T

# DVE bit-square instruction batches: maximal runs of consecutive DVE tiles,
# capped at 3 tiles per instruction
DVE_BATCHES = []
_run = []
for t in DVE_TILES:
    if _run and (t != _run[-1] + 1 or len(_run) == 3):
        DVE_BATCHES.append(_run)
        _run = []
    _run.append(t)
DVE_BATCHES.append(_run)

_cache = {}


def _np_dt(dt):
    return mybir.dt.np(dt)


def _build():
    nc = bacc.Bacc(trn_type="TRN2")
    z = nc.declare_dram_parameter("z", [P, T * FREE], Z_DT, isOutput=False)
    r8 = nc.declare_dram_parameter("r8", [P, NPAIRS * 2 * Q], Z_DT, isOutput=False)
    rb = nc.declare_dram_parameter("rb", [P, NPLAIN * Q], mybir.dt.bfloat16,
                                   isOutput=False)
    out = nc.declare_dram_parameter("out", [Q, 2 * FREE], mybir.dt.float32,
                                    isOutput=True)

    aslot = {t: j for j, t in enumerate(ACT_TILES)}   # tile -> sq8 slot
    dslot = {t: j for j, t in enumerate(DVE_TILES)}   # tile -> sqb slot

    with TileContext(nc) as tc:
        with (
            tc.tile_pool(name="singles", bufs=1) as singles,
            tc.tile_pool(name="ppool", bufs=1, space="PSUM") as ppool,
        ):
            r8_sb = singles.tile([P, NPAIRS, 2, Q], Z_DT)
            rb_sb = singles.tile([P, NPLAIN, Q], mybir.dt.bfloat16)
            nc.sync.dma_start(out=r8_sb[:], in_=r8[:])
            nc.sync.dma_start(out=rb_sb[:], in_=rb[:])

            z_sb = singles.tile([P, T, FREE], Z_DT)
            sq8_sb = singles.tile([P, 2 * NPAIRS, FREE], Z_DT)
            sqb_sb = singles.tile([P, NPLAIN, FREE], mybir.dt.bfloat16)

            t0 = 0
            for sz in SLAB_SIZES:
                nc.sync.dma_start(out=z_sb[:, t0:t0 + sz, :],
                                  in_=z[:, t0 * FREE:(t0 + sz) * FREE])
                t0 += sz

            # squares, emitted in arrival order
            units = []
            for k, (ta, tb) in enumerate(ACT_PAIRS):
                units.append((tb, "act", k))
            for bi, batch in enumerate(DVE_BATCHES):
                units.append((batch[-1], "dve", bi))
            units.sort()
            for _, kind, j in units:
                if kind == "act":
                    ta, tb = ACT_PAIRS[j]
                    nc.scalar.square(sq8_sb[:, 2 * j:2 * j + 2, :],
                                     z_sb[:, ta:tb + 1, :])
                else:
                    batch = DVE_BATCHES[j]
                    b0, b1 = batch[0], batch[-1] + 1
                    s0 = dslot[b0]
                    s1 = s0 + (b1 - b0)
                    nc.vector.tensor_scalar(
                        out=sqb_sb[:, s0:s1, :].bitcast(mybir.dt.uint16),
                        in0=z_sb[:, b0:b1, :].bitcast(mybir.dt.uint8),
                        scalar1=32.0, scalar2=SQ_K,
                        op0=mybir.AluOpType.mult, op1=mybir.AluOpType.add)

            acc_dr = [ppool.tile([Q, MM_N], mybir.dt.float32, name=f"adr{i}")
                      for i in range(NSLICES)]
            acc_pl = [ppool.tile([Q, MM_N], mybir.dt.float32, name=f"apl{i}")
                      for i in range(NSLICES)]

            mm_units = []
            for k, (ta, tb) in enumerate(ACT_PAIRS):
                mm_units.append((tb, "dr", k))
            for j, t in enumerate(DVE_TILES):
                mm_units.append((t, "pl", j))
            mm_units.sort()

            ndr = npl = 0
            for _, kind, j in mm_units:
                if kind == "dr":
                    for sl in range(NSLICES):
                        nc.tensor.matmul(
                            acc_dr[sl][:],
                            r8_sb[:, j, :, :],
                            sq8_sb[:, 2 * j:2 * j + 2, sl * MM_N:(sl + 1) * MM_N],
                            start=(ndr == 0),
                            stop=(ndr == NPAIRS - 1),
                            perf_mode=mybir.MatmulPerfMode.DoubleRow,
                        )
                    ndr += 1
                else:
                    for sl in range(NSLICES):
                        nc.tensor.matmul(
                            acc_pl[sl][:],
                            rb_sb[:, j, :],
                            sqb_sb[:, j, sl * MM_N:(sl + 1) * MM_N],
                            start=(npl == 0),
                            stop=(npl == NPLAIN - 1),
                        )
                    npl += 1

            out_sb = singles.tile([Q, 2 * FREE], mybir.dt.float32)
            for sl in range(NSLICES):
                copy_eng = nc.scalar.copy if sl % 2 == 0 else nc.vector.tensor_copy
                copy_eng(out_sb[:, sl * MM_N:(sl + 1) * MM_N], acc_dr[sl][:])
                copy_eng2 = nc.vector.tensor_copy if sl % 2 == 0 else nc.scalar.copy
                copy_eng2(out_sb[:, FREE + sl * MM_N:FREE + (sl + 1) * MM_N],
                          acc_pl[sl][:])
            nc.sync.dma_start(out=out[:], in_=out_sb[:])
    nc.compile()
    return nc


def _get_nc():
    if "nc" not in _cache:
        _cache["nc"] = _build()
    return _cache["nc"]


def _shard(Z, R):
    np_z = _np_dt(Z_DT)
    np_bf = _np_dt(mybir.dt.bfloat16)
    ZP = np.zeros((N_CORES * NC_ROWS, D), dtype=np_z)
    ZP[:N_FULL] = (np.abs(Z) * np.float32(Z_SCALE_IN)).astype(np_z)
    ZW = ZP.reshape(N_CORES, P, T * FREE)

    RP = np.zeros((N_CORES * NC_ROWS,), dtype=np.float32)
    RP[:N_FULL] = R
    RV = RP.reshape(N_CORES, P, T, Q)

    R8 = np.zeros((N_CORES, P, NPAIRS, 2, Q), dtype=np.float32)
    for k, (ta, tb) in enumerate(ACT_PAIRS):
        R8[:, :, k, 0] = RV[:, :, ta]
        R8[:, :, k, 1] = RV[:, :, tb]
    R8 = np.ascontiguousarray(R8.astype(np_z)).reshape(N_CORES, P, NPAIRS * 2 * Q)

    RB = np.zeros((N_CORES, P, NPLAIN, Q), dtype=np.float32)
    for j, t in enumerate(DVE_TILES):
        RB[:, :, j] = RV[:, :, t]
    RB = np.ascontiguousarray(RB.astype(np_bf)).reshape(N_CORES, P, NPLAIN * Q)

    return [{"z": ZW[kk], "r8": R8[kk], "rb": RB[kk]} for kk in range(N_CORES)]


def _combine(results):
    idx = np.arange(Q)
    s = 0.0
    for res in results:
        C = np.asarray(res["out"], dtype=np.float64).reshape(Q, 2, Q, D)
        s += C[idx, :, idx, :].sum()
    s /= float(Z_SCALE_IN) ** 2
    lam = np.exp(s)
    logits = 1.0 - np.exp(-lam)
    return np.float32(logits)


def _run(Z, R, trace=False, tmpdir=None):
    nc = _get_nc()
    in_maps = _shard(Z, R)
    return run_bass_kernel_spmd(nc, in_maps, core_ids=list(range(N_CORES)),
                                trace=trace, tmpdir=tmpdir)


def kernel(Z, R):
    assert Z.shape == (N_FULL, D) and R.shape == (N_FULL,)
    out = _run(np.asarray(Z), np.asarray(R), trace=False)
    return _combine(out.results)


# revision 11
# speedup vs baseline: 1.4194x; 1.0609x over previous
"""Trainium2 Bass kernel for nn_BPDecoder: logits = 1 - exp(-exp(sum_i R_i*||Z_i||^2)).

v5 (8-core SPMD, row-sharded, DVE bit-arithmetic squares):
  - The logits tolerance (2e-2) allows ~30% relative error on the scalar s,
    which unlocks aggressive quantization.  Z rides the wire as |Z|*512 in
    fp8 e4m3 (sign dropped -- only squares are needed); the result is
    divided by 512^2 on host.
  - Row r of a core maps to (partition, tile, q) = (r//496, (r%496)//16,
    r%16): the wire buffer is exactly Zfp8.reshape(128, 63488), and DMA
    slabs are per-partition contiguous runs of 1.4-8KB (measured
    ~385GB/s with big descriptors on the SP HWDGE ring).
  - 31 tiles, one resident SBUF wire buffer, two square pipelines:
      * 6 ACT pairs: nc.scalar.square fp8->fp8 (one 2-tile instruction per
        pair, ~1.85us/tile); PE contracts each pair with fp8 DoubleRow
        matmuls (256 rows, 2 cols/cycle at full clock).
      * 19 DVE tiles: *bit-arithmetic square* -- for abs-fp8 bytes,
        bf16_bits(z^2) ~= 32*byte + 14470 (exponent doubles, mantissa
        affine fit; measured s-error ~2% of budget).  This is a uint8
        tensor_scalar (mult+add) which runs in the DVE's dtype-agnostic
        2x_2P mode: ~1.15us/tile, half the cost of tensor_mul.  The
        uint16 outputs ARE bf16 squares; the PE consumes them with plain
        bf16 matmuls into a second PSUM group.
  - gpsimd never computes and never issues DMAs (Q7 activity trips a power
    throttle that halves DVE throughput, and SWDGE cast-DMAs double the
    DMA-engine byte load -- both measured).
  - Host extracts the q'==q diagonal blocks of both output groups and
    applies 1 - exp(-exp(s)) in f64.
"""

import sys

sys.path.insert(0, "/opt/trn_rl_repo")


def _install_ntff_hook_shim():
    import types
    if "antenv.axon_hooks" in sys.modules:
        return
    mod = types.ModuleType("antenv.axon_hooks")
    state = {"hook": None}
    mod.set_axon_ntff_profile_hook = lambda h: state.__setitem__("hook", h)
    mod.get_axon_ntff_profile_hook = lambda: state["hook"]
    sys.modules["antenv.axon_hooks"] = mod
    try:
        sys.path.insert(0, "/root/.axon_site")
        from trn_agent_boot.trn_boot import _ntff_profile_via_ctypes
        state["hook"] = _ntff_profile_via_ctypes("/opt/axon/libaxon_pjrt.so")
    except Exception:
        pass


_install_ntff_hook_shim()

import numpy as np

import concourse.bass as bass
import concourse.bacc as bacc
import concourse.mybir as mybir
from concourse.tile import TileContext
from concourse.bass_utils import run_bass_kernel_spmd

P = 128
D = 128
Q = 16
T = 31
FREE = Q * D                  # 2048
ROWS_PER_PART = T * Q         # 496
NC_ROWS = P * ROWS_PER_PART   # 63488
N_CORES = 8
N_FULL = 500000
MM_N = 512
NSLICES = FREE // MM_N        # 4

Z_DT = mybir.dt.float8e4
Z_SCALE_IN = 512.0
SQ_K = 14470.0                # bf16_bits(z^2) ~= 32*abs_fp8_byte + SQ_K

# ACT squares these adjacent tile pairs (fp8 -> DoubleRow matmuls) plus one
# early single tile (plain fp8 matmuls); DVE bit-squares the rest (bf16 ->
# plain matmuls).  Pairs are placed so each arrives just in time for ACT's
# ~3.7us/pair cadence, and the last pair/batch of each engine lands as the
# DMA stream ends so both engines finish together.
ACT_PAIRS = [(0, 1), (4, 5), (9, 10), (14, 15), (19, 20), (27, 28)]
ACT_SINGLES = [8]
ACT_TILES = [t for p in ACT_PAIRS for t in p] + ACT_SINGLES
DVE_TILES = [t for t in range(T) if t not in ACT_TILES]
NPAIRS = len(ACT_PAIRS)       # 6
NSING = len(ACT_SINGLES)      # 1
NPLAIN = len(DVE_TILES)       # 18

SLAB_SIZES = [2, 4, 5, 5, 5, 5, 3, 2]
assert sum(SLAB_SIZES) == T

# DVE bit-square instruction batches: maximal runs of consecutive DVE tiles,
# capped at 3 tiles per instruction
DVE_BATCHES = []
_run = []
for t in DVE_TILES:
    if _run and (t != _run[-1] + 1 or len(_run) == 3):
        DVE_BATCHES.append(_run)
        _run = []
    _run.append(t)
DVE_BATCHES.append(_run)

_cache = {}


def _np_dt(dt):
    return mybir.dt.np(dt)


def _build():
    nc = bacc.Bacc(trn_type="TRN2")
    z = nc.declare_dram_parameter("z", [P, T * FREE], Z_DT, isOutput=False)
    r8 = nc.declare_dram_parameter("r8", [P, (NPAIRS * 2 + NSING) * Q], Z_DT,
                                   isOutput=False)
    rb = nc.declare_dram_parameter("rb", [P, NPLAIN * Q], mybir.dt.bfloat16,
                                   isOutput=False)
    out = nc.declare_dram_parameter("out", [Q, FREE], mybir.dt.float32,
                                    isOutput=True)

    dslot = {t: j for j, t in enumerate(DVE_TILES)}   # tile -> sqb slot

    with TileContext(nc) as tc:
        with (
            tc.tile_pool(name="singles", bufs=1) as singles,
            tc.tile_pool(name="ppool", bufs=1, space="PSUM") as ppool,
        ):
            r8_sb = singles.tile([P, NPAIRS * 2 + NSING, Q], Z_DT)
            rb_sb = singles.tile([P, NPLAIN, Q], mybir.dt.bfloat16)
            nc.sync.dma_start(out=r8_sb[:], in_=r8[:])
            nc.sync.dma_start(out=rb_sb[:], in_=rb[:])

            z_sb = singles.tile([P, T, FREE], Z_DT)
            sq8_sb = singles.tile([P, 2 * NPAIRS + NSING, FREE], Z_DT)
            sqb_sb = singles.tile([P, NPLAIN, FREE], mybir.dt.bfloat16)

            t0 = 0
            for sz in SLAB_SIZES:
                nc.sync.dma_start(out=z_sb[:, t0:t0 + sz, :],
                                  in_=z[:, t0 * FREE:(t0 + sz) * FREE])
                t0 += sz

            # squares, emitted in arrival order
            units = []
            for k, (ta, tb) in enumerate(ACT_PAIRS):
                units.append((tb, "act", k))
            for si, t in enumerate(ACT_SINGLES):
                units.append((t, "as", si))
            for bi, batch in enumerate(DVE_BATCHES):
                units.append((batch[-1], "dve", bi))
            units.sort()
            for _, kind, j in units:
                if kind == "act":
                    ta, tb = ACT_PAIRS[j]
                    nc.scalar.square(sq8_sb[:, 2 * j:2 * j + 2, :],
                                     z_sb[:, ta:tb + 1, :])
                elif kind == "as":
                    t = ACT_SINGLES[j]
                    nc.scalar.square(sq8_sb[:, 2 * NPAIRS + j, :], z_sb[:, t, :])
                else:
                    batch = DVE_BATCHES[j]
                    b0, b1 = batch[0], batch[-1] + 1
                    s0 = dslot[b0]
                    s1 = s0 + (b1 - b0)
                    nc.vector.tensor_scalar(
                        out=sqb_sb[:, s0:s1, :].bitcast(mybir.dt.uint16),
                        in0=z_sb[:, b0:b1, :].bitcast(mybir.dt.uint8),
                        scalar1=32.0, scalar2=SQ_K,
                        op0=mybir.AluOpType.mult, op1=mybir.AluOpType.add)

            # one merged accumulation group: DoubleRow pairs, the fp8 single
            # and the bf16 tiles all accumulate into the same 4 PSUM banks
            accs = [ppool.tile([Q, MM_N], mybir.dt.float32, name=f"acc{i}")
                    for i in range(NSLICES)]

            mm_units = []
            for k, (ta, tb) in enumerate(ACT_PAIRS):
                mm_units.append((tb, "dr", k))
            for si, t in enumerate(ACT_SINGLES):
                mm_units.append((t, "s8", si))
            for j, t in enumerate(DVE_TILES):
                mm_units.append((t, "pl", j))
            mm_units.sort()

            nmm = 0
            nunits = len(mm_units)
            for _, kind, j in mm_units:
                start = (nmm == 0)
                stop = (nmm == nunits - 1)
                if kind == "dr":
                    for sl in range(NSLICES):
                        nc.tensor.matmul(
                            accs[sl][:],
                            r8_sb[:, 2 * j:2 * j + 2, :],
                            sq8_sb[:, 2 * j:2 * j + 2, sl * MM_N:(sl + 1) * MM_N],
                            start=start, stop=stop,
                            perf_mode=mybir.MatmulPerfMode.DoubleRow,
                        )
                elif kind == "s8":
                    for sl in range(NSLICES):
                        nc.tensor.matmul(
                            accs[sl][:],
                            r8_sb[:, 2 * NPAIRS + j, :],
                            sq8_sb[:, 2 * NPAIRS + j, sl * MM_N:(sl + 1) * MM_N],
                            start=start, stop=stop,
                        )
                else:
                    for sl in range(NSLICES):
                        nc.tensor.matmul(
                            accs[sl][:],
                            rb_sb[:, j, :],
                            sqb_sb[:, j, sl * MM_N:(sl + 1) * MM_N],
                            start=start, stop=stop,
                        )
                nmm += 1

            out_sb = singles.tile([Q, FREE], mybir.dt.float32)
            for sl in range(NSLICES):
                copy_eng = nc.scalar.copy if sl % 2 == 0 else nc.vector.tensor_copy
                copy_eng(out_sb[:, sl * MM_N:(sl + 1) * MM_N], accs[sl][:])
            nc.sync.dma_start(out=out[:], in_=out_sb[:])
    nc.compile()
    return nc


def _get_nc():
    if "nc" not in _cache:
        _cache["nc"] = _build()
    return _cache["nc"]


def _shard(Z, R):
    np_z = _np_dt(Z_DT)
    np_bf = _np_dt(mybir.dt.bfloat16)
    ZP = np.zeros((N_CORES * NC_ROWS, D), dtype=np_z)
    ZP[:N_FULL] = (np.abs(Z) * np.float32(Z_SCALE_IN)).astype(np_z)
    ZW = ZP.reshape(N_CORES, P, T * FREE)

    RP = np.zeros((N_CORES * NC_ROWS,), dtype=np.float32)
    RP[:N_FULL] = R
    RV = RP.reshape(N_CORES, P, T, Q)

    R8 = np.zeros((N_CORES, P, NPAIRS * 2 + NSING, Q), dtype=np.float32)
    for k, (ta, tb) in enumerate(ACT_PAIRS):
        R8[:, :, 2 * k] = RV[:, :, ta]
        R8[:, :, 2 * k + 1] = RV[:, :, tb]
    for si, t in enumerate(ACT_SINGLES):
        R8[:, :, 2 * NPAIRS + si] = RV[:, :, t]
    R8 = np.ascontiguousarray(R8.astype(np_z)).reshape(
        N_CORES, P, (NPAIRS * 2 + NSING) * Q)

    RB = np.zeros((N_CORES, P, NPLAIN, Q), dtype=np.float32)
    for j, t in enumerate(DVE_TILES):
        RB[:, :, j] = RV[:, :, t]
    RB = np.ascontiguousarray(RB.astype(np_bf)).reshape(N_CORES, P, NPLAIN * Q)

    return [{"z": ZW[kk], "r8": R8[kk], "rb": RB[kk]} for kk in range(N_CORES)]


def _combine(results):
    idx = np.arange(Q)
    s = 0.0
    for res in results:
        C = np.asarray(res["out"], dtype=np.float64).reshape(Q, Q, D)
        s += C[idx, idx, :].sum()
    s /= float(Z_SCALE_IN) ** 2
    lam = np.exp(s)
    logits = 1.0 - np.exp(-lam)
    return np.float32(logits)


def _run(Z, R, trace=False, tmpdir=None):
    nc = _get_nc()
    in_maps = _shard(Z, R)
    return run_bass_kernel_spmd(nc, in_maps, core_ids=list(range(N_CORES)),
                                trace=trace, tmpdir=tmpdir)


def kernel(Z, R):
    assert Z.shape == (N_FULL, D) and R.shape == (N_FULL,)
    out = _run(np.asarray(Z), np.asarray(R), trace=False)
    return _combine(out.results)
